# revision 13
# baseline (speedup 1.0000x reference)
"""Trainium2 Bass kernel for nn_MultiHeadGraphAttention (v2).

Multi-head graph attention (GAT-style):
    h_prime = einsum('nf,hfo->hno', h, w)
    attn    = softmax(where(adj, leakyrelu(s_i + d_j), -inf), axis=-1)
    out     = attn @ h_prime + b
with s = h_prime @ a_src, d = h_prime @ a_dst, n=4096, H=8, f_out=64.

Key identity:  exp(leakyrelu(x)) = e^{0.2x} * max(e^{0.8x}, 1)
For x = s_i + d_j the e^{0.2x} factor splits into e^{0.2 s_i} (per attention
row i => cancels in the softmax normalization, done on host) and e^{0.2 d_j}
(per contraction index j => folded into the matmul stationary operand
Vt = [V*v2 | v2] on host).  The device therefore only computes, per j-chunk,
    p[j,i] = max(e^{0.8(s_i+d_j)}, 1) * m[j,i]
and accumulates out^T = Vt^T @ p in PSUM (the ones column of Vt yields the
softmax denominators for free).  Final normalize + transpose on host.

Sharding: 8 cores = 4 head-pairs x 2 column-halves.  Each core computes two
heads over a 2048-wide slice of attention rows i; the adjacency mask slice
(bf16 [4096, 2048]) is shared by both heads and DMA'd once per j-chunk.

Per (head, j-chunk) slot, one of three balanced routes computes p:
  V: DVE  tensor_scalar 4x  t = (U8 * v8_j) max 1 ;  p = t * m      (TT 2x)
  A: ACT  r = Relu(S_b + d_j);  e = Exp(0.8 r)     ;  p = e * m      (TT 2x)
  M: ACT  E = Exp(0.8 SM + 0.8 d_j)  (SM host-masked to -300)
          p = max(E, m)   -- masked: E~0, m=0 -> 0; else max(e^{0.8x},1)
with U8 = e^{0.8 s} broadcast, v8_j = e^{0.8 d_j}, all host-precomputed.
"""
import sys

if "/opt/trn_rl_repo" not in sys.path:
    sys.path.insert(0, "/opt/trn_rl_repo")

from contextlib import ExitStack

import ml_dtypes
import numpy as np

import concourse.bass as bass
import concourse.bacc as bacc
import concourse.tile as tile
from concourse import mybir
from concourse.bass_utils import run_bass_kernel_spmd

F32 = mybir.dt.float32
BF16 = mybir.dt.bfloat16
AF = mybir.ActivationFunctionType
ALU = mybir.AluOpType

N = 4096
F_IN = 256
N_HEAD = 8
F_OUT = 64
NEG = 0.2
W = 2048              # attention-row (i) slice width per core
NCH = N // 128        # 32 j-chunks
VW = F_OUT + 1        # 65: V columns + ones (denominator) column
NSLOT = 2 * NCH       # 64 (head, chunk) slots, k = 2*jc + h
MASKED = -3e5  # huge-negative marker: survives M2's Prelu(alpha=1e-4) kink


def _mk_routes(counts):
    assert sum(counts.values()) == NSLOT
    acc = {k: 0.0 for k in counts}
    out = []
    for _ in range(NSLOT):
        for k in acc:
            acc[k] += counts[k] / NSLOT
        k = max(acc, key=lambda q: (acc[q], q))
        acc[k] -= 1.0
        out.append(k)
    # fast ramp: first two slots want cheap DVE chunks
    for i in range(2):
        if out[i] not in ("V", "VG"):
            j = next(j for j in range(2, NSLOT) if out[j] == "V")
            out[i], out[j] = out[j], out[i]
    return out


# V: DVE TS+TT | VG: TS on DVE, TT-mult on GpSimd | VG2: TS on GpSimd, TT on
# DVE | A: 2xACT + TT | M: ACT(SM)+TTmax | M2: ACT Prelu(SM,a=1e-4)+Exp, no TT
# (gpsimd TT op=max and gpsimd STT fail walrus lowering - do not use)
ROUTE_COUNTS = {"V": 14, "VG": 12, "VG2": 12, "M": 16, "M2": 4, "A": 6}
ROUTES = _mk_routes(ROUTE_COUNTS)
# SM tensor blocks needed by M-family routes, in emission order
SM_ROUTES = ("M", "MG", "M2")
N_M = sum(r in SM_ROUTES for r in ROUTES)


def build_program(routes=ROUTES):
    n_m = sum(r in SM_ROUTES for r in routes)
    nc = bacc.Bacc("TRN2", target_bir_lowering=False, debug=False)
    maskT = nc.dram_tensor("maskT", [N, W], BF16, kind="ExternalInput").ap()
    u8b = [nc.dram_tensor(f"u8b{h}", [128, W], BF16, kind="ExternalInput").ap()
           for h in range(2)]
    sbb = [nc.dram_tensor(f"sbb{h}", [128, W], BF16, kind="ExternalInput").ap()
           for h in range(2)]
    vt = [nc.dram_tensor(f"vt{h}", [128, NCH * VW], BF16, kind="ExternalInput").ap()
          for h in range(2)]
    # per-chunk per-partition scalars: [:, jc] columns
    v8c = [nc.dram_tensor(f"v8c{h}", [128, NCH], F32, kind="ExternalInput").ap()
           for h in range(2)]
    dc = [nc.dram_tensor(f"dc{h}", [128, NCH], F32, kind="ExternalInput").ap()
          for h in range(2)]
    d8c = [nc.dram_tensor(f"d8c{h}", [128, NCH], F32, kind="ExternalInput").ap()
           for h in range(2)]
    smm = nc.dram_tensor("smm", [max(n_m, 1) * 128, W], BF16,
                         kind="ExternalInput").ap()
    outT = [nc.dram_tensor(f"outT{h}", [VW, W], F32, kind="ExternalOutput").ap()
            for h in range(2)]

    with tile.TileContext(nc) as tc, ExitStack() as ctx:
        const_pool = ctx.enter_context(tc.tile_pool(name="const", bufs=1))
        mask_pool = ctx.enter_context(tc.tile_pool(name="mask", bufs=6))
        sm_pool = ctx.enter_context(tc.tile_pool(name="sm", bufs=4))
        t_pool = ctx.enter_context(tc.tile_pool(name="tw", bufs=3))
        r_pool = ctx.enter_context(tc.tile_pool(name="rw", bufs=2))
        e_pool = ctx.enter_context(tc.tile_pool(name="ew", bufs=4))
        p_pool = ctx.enter_context(tc.tile_pool(name="pw", bufs=6))
        ps_pool = ctx.enter_context(tc.tile_pool(name="ps", bufs=1, space="PSUM"))

        # ---- ramp: prefetch first mask chunks before the const bulk ----
        PRE = 4
        mask_tiles = {}
        for jc in range(PRE):
            m_t = mask_pool.tile([128, W], BF16, tag="mt", name=f"mpre{jc}")
            nc.sync.dma_start(m_t[:, :], maskT[jc * 128:(jc + 1) * 128, :])
            mask_tiles[jc] = m_t

        # ---- constants (h0 first so its chunks unblock early) ----
        u8_sb, sb_sb, vt_sb, v8_sb, dc_sb, d8_sb = [], [], [], [], [], []
        for h in range(2):
            u8t = const_pool.tile([128, W], BF16, tag=f"u8_{h}")
            nc.sync.dma_start(u8t[:, :], u8b[h][:, :])
            u8_sb.append(u8t)
            v8t = const_pool.tile([128, NCH], F32, tag=f"v8_{h}")
            nc.sync.dma_start(v8t[:, :], v8c[h][:, :])
            v8_sb.append(v8t)
            vtt = const_pool.tile([128, NCH * VW], BF16, tag=f"vt_{h}")
            # split into 4 DMAs so early chunks' stationaries land fast
            Q = NCH * VW // 4
            for q in range(4):
                nc.sync.dma_start(vtt[:, q * Q:(q + 1) * Q],
                                  vt[h][:, q * Q:(q + 1) * Q])
            vt_sb.append(vtt)
            dct = const_pool.tile([128, NCH], F32, tag=f"dc_{h}")
            nc.sync.dma_start(dct[:, :], dc[h][:, :])
            dc_sb.append(dct)
            d8t = const_pool.tile([128, NCH], F32, tag=f"d8_{h}")
            nc.sync.dma_start(d8t[:, :], d8c[h][:, :])
            d8_sb.append(d8t)
        for h in range(2):
            sbt = const_pool.tile([128, W], BF16, tag=f"sb_{h}")
            nc.sync.dma_start(sbt[:, :], sbb[h][:, :])
            sb_sb.append(sbt)

        ps_O = [ps_pool.tile([VW, W], F32, tag=f"psO{h}", name=f"psO{h}")
                for h in range(2)]

        # ---- attention j-loop (jc-major, heads inner; mask loaded once/jc) --
        mi = 0  # M-family block counter into smm
        for jc in range(NCH):
            if jc in mask_tiles:
                m_t = mask_tiles[jc]
            else:
                m_t = mask_pool.tile([128, W], BF16, tag="mt")
                nc.sync.dma_start(m_t[:, :], maskT[jc * 128:(jc + 1) * 128, :])
            for h in range(2):
                k = 2 * jc + h
                r = routes[k]
                p_t = p_pool.tile([128, W], BF16, tag="pt")
                if r in ("V", "VG", "VG2"):
                    t_t = t_pool.tile([128, W], BF16, tag="tt")
                    ts_eng = nc.gpsimd if r == "VG2" else nc.vector
                    ts_eng.tensor_scalar(t_t[:, :], u8_sb[h][:, :],
                                         v8_sb[h][:, jc:jc + 1], 1.0,
                                         op0=ALU.mult, op1=ALU.max)
                    tt_eng = nc.gpsimd if r == "VG" else nc.vector
                    tt_eng.tensor_tensor(p_t[:, :], t_t[:, :], m_t[:, :],
                                         op=ALU.mult)
                elif r == "A":
                    r_t = r_pool.tile([128, W], F32, tag="rt")
                    nc.scalar.activation(r_t[:, :], sb_sb[h][:, :], AF.Relu,
                                         bias=dc_sb[h][:, jc:jc + 1])
                    e_t = e_pool.tile([128, W], BF16, tag="et")
                    nc.scalar.activation(e_t[:, :], r_t[:, :], AF.Exp,
                                         scale=0.8)
                    nc.vector.tensor_tensor(p_t[:, :], e_t[:, :], m_t[:, :],
                                            op=ALU.mult)
                elif r in ("M", "MG"):
                    sm_t = sm_pool.tile([128, W], BF16, tag="smt")
                    nc.sync.dma_start(sm_t[:, :],
                                      smm[mi * 128:(mi + 1) * 128, :])
                    mi += 1
                    e_t = e_pool.tile([128, W], BF16, tag="et")
                    nc.scalar.activation(e_t[:, :], sm_t[:, :], AF.Exp,
                                         bias=d8_sb[h][:, jc:jc + 1], scale=0.8)
                    eng = nc.gpsimd if r == "MG" else nc.vector
                    eng.tensor_tensor(p_t[:, :], e_t[:, :], m_t[:, :],
                                      op=ALU.max)
                else:  # "M2": Prelu(alpha~0) keeps the masked marker; no TT
                    sm_t = sm_pool.tile([128, W], BF16, tag="smt")
                    nc.sync.dma_start(sm_t[:, :],
                                      smm[mi * 128:(mi + 1) * 128, :])
                    mi += 1
                    r_t = r_pool.tile([128, W], F32, tag="rt")
                    nc.scalar.activation(r_t[:, :], sm_t[:, :], AF.Prelu,
                                         bias=dc_sb[h][:, jc:jc + 1],
                                         alpha=1e-4)
                    nc.scalar.activation(p_t[:, :], r_t[:, :], AF.Exp,
                                         scale=0.8)
                for q in range(W // 512):
                    nc.tensor.matmul(ps_O[h][:, q * 512:(q + 1) * 512],
                                     vt_sb[h][:, jc * VW:(jc + 1) * VW],
                                     p_t[:, q * 512:(q + 1) * 512],
                                     start=(jc == 0), stop=(jc == NCH - 1))

        for h in range(2):
            o_t = const_pool.tile([VW, W], F32, tag=f"ot{h}", name=f"ot{h}")
            nc.scalar.copy(o_t[:, :], ps_O[h][:, :])
            nc.sync.dma_start(outT[h][:, :], o_t[:, :])
    nc.compile()
    return nc


_CACHED_NC = None


def _get_nc():
    global _CACHED_NC
    if _CACHED_NC is None:
        _CACHED_NC = build_program()
    return _CACHED_NC


def _bf(x):
    return np.ascontiguousarray(x.astype(ml_dtypes.bfloat16))


def _prep_inputs(h, adj, w, a_src, a_dst, b):
    h = np.asarray(h, dtype=np.float32)
    adj = np.asarray(adj)
    w = np.asarray(w, dtype=np.float32)
    a_src = np.asarray(a_src, dtype=np.float32)
    a_dst = np.asarray(a_dst, dtype=np.float32)
    b = np.asarray(b, dtype=np.float32)

    adjT = adj.T  # [j, i] layout
    # per global head: s, d, V
    s_all, d_all, vt_all = [], [], []
    for g in range(N_HEAD):
        s = h @ (w[g] @ a_src[g])[:, 0]             # [N] per-row logit
        d = h @ (w[g] @ a_dst[g])[:, 0]             # [N] per-col logit
        V = h @ w[g] + b[None, :]                   # [N, F_OUT]
        v2 = np.exp(NEG * d)                        # e^{0.2 d}
        vt = np.concatenate([V * v2[:, None], v2[:, None]], axis=1)  # [N, VW]
        s_all.append(s)
        d_all.append(d)
        vt_all.append(vt)

    in_maps = []
    for c in range(N_HEAD):
        pair, half = c % 4, c // 4
        isl = slice(half * W, (half + 1) * W)
        adjT_sl = adjT[:, isl]                      # [N, W] bool
        mp = {"maskT": _bf(adjT_sl.astype(np.float32))}
        smm_blocks = []
        for hh in range(2):
            g = 2 * pair + hh
            s = s_all[g]
            d = d_all[g]
            s_sl = s[isl].astype(np.float32)
            mp[f"u8b{hh}"] = _bf(np.broadcast_to(
                np.exp(0.8 * s_sl)[None, :], (128, W)))
            mp[f"sbb{hh}"] = _bf(np.broadcast_to(s_sl[None, :], (128, W)))
            vt128 = vt_all[g].reshape(NCH, 128, VW).transpose(1, 0, 2)
            mp[f"vt{hh}"] = _bf(vt128.reshape(128, NCH * VW))
            dcol = d.reshape(NCH, 128).T.astype(np.float32)     # [128, NCH]
            mp[f"v8c{hh}"] = np.ascontiguousarray(np.exp(0.8 * dcol))
            mp[f"dc{hh}"] = np.ascontiguousarray(dcol)
            mp[f"d8c{hh}"] = np.ascontiguousarray(0.8 * dcol)
        # SM blocks for M-route slots, in device emission order
        s_bf_sl = [None, None]
        for k, r in enumerate(ROUTES):
            if r not in SM_ROUTES:
                continue
            jc, hh = k // 2, k % 2
            g = 2 * pair + hh
            if s_bf_sl[hh] is None:
                s_bf_sl[hh] = s_all[g][isl].astype(np.float32)
            blk = np.where(adjT_sl[jc * 128:(jc + 1) * 128, :],
                           s_bf_sl[hh][None, :], np.float32(MASKED))
            smm_blocks.append(blk)
        if smm_blocks:
            mp["smm"] = _bf(np.concatenate(smm_blocks, axis=0))
        else:
            mp["smm"] = _bf(np.zeros((128, W), np.float32))
        in_maps.append(mp)
    return in_maps


def _run(in_maps, trace=False, **kwargs):
    nc = _get_nc()
    return run_bass_kernel_spmd(nc, in_maps, list(range(N_HEAD)), trace=trace,
                                **kwargs)


def _assemble(res):
    out = np.empty((N_HEAD, N, F_OUT), dtype=np.float32)
    for c in range(N_HEAD):
        pair, half = c % 4, c // 4
        isl = slice(half * W, (half + 1) * W)
        for hh in range(2):
            g = 2 * pair + hh
            blk = np.asarray(res.results[c][f"outT{hh}"], dtype=np.float32)
            out[g, isl, :] = (blk[:F_OUT, :] / blk[F_OUT:VW, :]).T
    return out


def kernel(h, adj, w, a_src, a_dst, b):
    in_maps = _prep_inputs(h, adj, w, a_src, a_dst, b)
    res = _run(in_maps)
    return _assemble(res)


# revision 14
# speedup vs baseline: 3.7382x; 3.7382x over previous
"""Trainium2 Bass kernel for nn_MultiHeadGraphAttention (v2).

Multi-head graph attention (GAT-style):
    h_prime = einsum('nf,hfo->hno', h, w)
    attn    = softmax(where(adj, leakyrelu(s_i + d_j), -inf), axis=-1)
    out     = attn @ h_prime + b
with s = h_prime @ a_src, d = h_prime @ a_dst, n=4096, H=8, f_out=64.

Key identity:  exp(leakyrelu(x)) = e^{0.2x} * max(e^{0.8x}, 1)
For x = s_i + d_j the e^{0.2x} factor splits into e^{0.2 s_i} (per attention
row i => cancels in the softmax normalization, done on host) and e^{0.2 d_j}
(per contraction index j => folded into the matmul stationary operand
Vt = [V*v2 | v2] on host).  The device therefore only computes, per j-chunk,
    p[j,i] = max(e^{0.8(s_i+d_j)}, 1) * m[j,i]
and accumulates out^T = Vt^T @ p in PSUM (the ones column of Vt yields the
softmax denominators for free).  Final normalize + transpose on host.

Sharding: 8 cores = 4 head-pairs x 2 column-halves.  Each core computes two
heads over a 2048-wide slice of attention rows i; the adjacency mask slice
(bf16 [4096, 2048]) is shared by both heads and DMA'd once per j-chunk.

Per (head, j-chunk) slot, one of three balanced routes computes p:
  V: DVE  tensor_scalar 4x  t = (U8 * v8_j) max 1 ;  p = t * m      (TT 2x)
  A: ACT  r = Relu(S_b + d_j);  e = Exp(0.8 r)     ;  p = e * m      (TT 2x)
  M: ACT  E = Exp(0.8 SM + 0.8 d_j)  (SM host-masked to -300)
          p = max(E, m)   -- masked: E~0, m=0 -> 0; else max(e^{0.8x},1)
with U8 = e^{0.8 s} broadcast, v8_j = e^{0.8 d_j}, all host-precomputed.
"""
import sys

if "/opt/trn_rl_repo" not in sys.path:
    sys.path.insert(0, "/opt/trn_rl_repo")

from contextlib import ExitStack

import ml_dtypes
import numpy as np

import concourse.bass as bass
import concourse.bacc as bacc
import concourse.tile as tile
from concourse import mybir
from concourse.bass_utils import run_bass_kernel_spmd

F32 = mybir.dt.float32
BF16 = mybir.dt.bfloat16
AF = mybir.ActivationFunctionType
ALU = mybir.AluOpType

N = 4096
F_IN = 256
N_HEAD = 8
F_OUT = 64
NEG = 0.2
W = 2048              # attention-row (i) slice width per core
NCH = N // 128        # 32 j-chunks
VW = F_OUT + 1        # 65: V columns + ones (denominator) column
NSLOT = 2 * NCH       # 64 (head, chunk) slots, k = 2*jc + h
MASKED = -3e5  # huge-negative marker: survives M2's Prelu(alpha=1e-4) kink


def _mk_routes(counts):
    assert sum(counts.values()) == NSLOT
    acc = {k: 0.0 for k in counts}
    out = []
    for _ in range(NSLOT):
        for k in acc:
            acc[k] += counts[k] / NSLOT
        k = max(acc, key=lambda q: (acc[q], q))
        acc[k] -= 1.0
        out.append(k)
    # fast ramp: first two slots want cheap DVE chunks
    for i in range(2):
        if out[i] not in ("V", "VG"):
            j = next(j for j in range(2, NSLOT) if out[j] == "V")
            out[i], out[j] = out[j], out[i]
    return out


# V: DVE TS+TT | A: 2xACT + TT | M: ACT(SM)+TTmax | M2: ACT Prelu(SM,a=1e-4)
# +Exp, no TT, no mask read.  GpSimd is unusable: TS measured 30us/op and its
# SBUF-port contention slows concurrent DVE ops 4-13x; TT(max)/STT don't
# even lower in walrus.
ROUTE_COUNTS = {"V": 35, "M": 13, "M2": 10, "A": 6}
ROUTES = _mk_routes(ROUTE_COUNTS)
# SM tensor blocks needed by M-family routes, in emission order
SM_ROUTES = ("M", "MG", "M2")
N_M = sum(r in SM_ROUTES for r in ROUTES)


def build_program(routes=ROUTES):
    n_m = sum(r in SM_ROUTES for r in routes)
    nc = bacc.Bacc("TRN2", target_bir_lowering=False, debug=False)
    maskT = nc.dram_tensor("maskT", [N, W], BF16, kind="ExternalInput").ap()
    u8b = [nc.dram_tensor(f"u8b{h}", [128, W], BF16, kind="ExternalInput").ap()
           for h in range(2)]
    sbb = [nc.dram_tensor(f"sbb{h}", [128, W], BF16, kind="ExternalInput").ap()
           for h in range(2)]
    vt = [nc.dram_tensor(f"vt{h}", [128, NCH * VW], BF16, kind="ExternalInput").ap()
          for h in range(2)]
    # per-chunk per-partition scalars: [:, jc] columns
    v8c = [nc.dram_tensor(f"v8c{h}", [128, NCH], F32, kind="ExternalInput").ap()
           for h in range(2)]
    dc = [nc.dram_tensor(f"dc{h}", [128, NCH], F32, kind="ExternalInput").ap()
          for h in range(2)]
    d8c = [nc.dram_tensor(f"d8c{h}", [128, NCH], F32, kind="ExternalInput").ap()
           for h in range(2)]
    smm = nc.dram_tensor("smm", [max(n_m, 1) * 128, W], BF16,
                         kind="ExternalInput").ap()
    outT = [nc.dram_tensor(f"outT{h}", [VW, W], F32, kind="ExternalOutput").ap()
            for h in range(2)]

    with tile.TileContext(nc) as tc, ExitStack() as ctx:
        const_pool = ctx.enter_context(tc.tile_pool(name="const", bufs=1))
        mask_pool = ctx.enter_context(tc.tile_pool(name="mask", bufs=6))
        sm_pool = ctx.enter_context(tc.tile_pool(name="sm", bufs=4))
        t_pool = ctx.enter_context(tc.tile_pool(name="tw", bufs=3))
        r_pool = ctx.enter_context(tc.tile_pool(name="rw", bufs=2))
        e_pool = ctx.enter_context(tc.tile_pool(name="ew", bufs=4))
        p_pool = ctx.enter_context(tc.tile_pool(name="pw", bufs=6))
        ps_pool = ctx.enter_context(tc.tile_pool(name="ps", bufs=1, space="PSUM"))

        # ---- ramp: prefetch first mask chunks before the const bulk ----
        PRE = 4
        mask_tiles = {}
        for jc in range(PRE):
            m_t = mask_pool.tile([128, W], BF16, tag="mt", name=f"mpre{jc}")
            nc.sync.dma_start(m_t[:, :], maskT[jc * 128:(jc + 1) * 128, :])
            mask_tiles[jc] = m_t

        # ---- constants (h0 first so its chunks unblock early) ----
        u8_sb, sb_sb, vt_sb, v8_sb, dc_sb, d8_sb = [], [], [], [], [], []
        for h in range(2):
            u8t = const_pool.tile([128, W], BF16, tag=f"u8_{h}")
            nc.sync.dma_start(u8t[:, :], u8b[h][:, :])
            u8_sb.append(u8t)
            v8t = const_pool.tile([128, NCH], F32, tag=f"v8_{h}")
            nc.sync.dma_start(v8t[:, :], v8c[h][:, :])
            v8_sb.append(v8t)
            vtt = const_pool.tile([128, NCH * VW], BF16, tag=f"vt_{h}")
            # split into 4 DMAs so early chunks' stationaries land fast
            Q = NCH * VW // 4
            for q in range(4):
                nc.sync.dma_start(vtt[:, q * Q:(q + 1) * Q],
                                  vt[h][:, q * Q:(q + 1) * Q])
            vt_sb.append(vtt)
            dct = const_pool.tile([128, NCH], F32, tag=f"dc_{h}")
            nc.sync.dma_start(dct[:, :], dc[h][:, :])
            dc_sb.append(dct)
            d8t = const_pool.tile([128, NCH], F32, tag=f"d8_{h}")
            nc.sync.dma_start(d8t[:, :], d8c[h][:, :])
            d8_sb.append(d8t)
        for h in range(2):
            sbt = const_pool.tile([128, W], BF16, tag=f"sb_{h}")
            nc.sync.dma_start(sbt[:, :], sbb[h][:, :])
            sb_sb.append(sbt)

        ps_O = [ps_pool.tile([VW, W], F32, tag=f"psO{h}", name=f"psO{h}")
                for h in range(2)]

        # ---- attention j-loop (jc-major, heads inner; mask loaded once/jc) --
        mi = 0  # M-family block counter into smm
        for jc in range(NCH):
            if jc in mask_tiles:
                m_t = mask_tiles[jc]
            else:
                m_t = mask_pool.tile([128, W], BF16, tag="mt")
                nc.sync.dma_start(m_t[:, :], maskT[jc * 128:(jc + 1) * 128, :])
            for h in range(2):
                k = 2 * jc + h
                r = routes[k]
                p_t = p_pool.tile([128, W], BF16, tag="pt")
                if r in ("V", "VG", "VG2"):
                    t_t = t_pool.tile([128, W], BF16, tag="tt")
                    ts_eng = nc.gpsimd if r == "VG2" else nc.vector
                    ts_eng.tensor_scalar(t_t[:, :], u8_sb[h][:, :],
                                         v8_sb[h][:, jc:jc + 1], 1.0,
                                         op0=ALU.mult, op1=ALU.max)
                    tt_eng = nc.gpsimd if r == "VG" else nc.vector
                    tt_eng.tensor_tensor(p_t[:, :], t_t[:, :], m_t[:, :],
                                         op=ALU.mult)
                elif r == "A":
                    r_t = r_pool.tile([128, W], F32, tag="rt")
                    nc.scalar.activation(r_t[:, :], sb_sb[h][:, :], AF.Relu,
                                         bias=dc_sb[h][:, jc:jc + 1])
                    e_t = e_pool.tile([128, W], BF16, tag="et")
                    nc.scalar.activation(e_t[:, :], r_t[:, :], AF.Exp,
                                         scale=0.8)
                    nc.vector.tensor_tensor(p_t[:, :], e_t[:, :], m_t[:, :],
                                            op=ALU.mult)
                elif r in ("M", "MG"):
                    sm_t = sm_pool.tile([128, W], BF16, tag="smt")
                    nc.sync.dma_start(sm_t[:, :],
                                      smm[mi * 128:(mi + 1) * 128, :])
                    mi += 1
                    e_t = e_pool.tile([128, W], BF16, tag="et")
                    nc.scalar.activation(e_t[:, :], sm_t[:, :], AF.Exp,
                                         bias=d8_sb[h][:, jc:jc + 1], scale=0.8)
                    eng = nc.gpsimd if r == "MG" else nc.vector
                    eng.tensor_tensor(p_t[:, :], e_t[:, :], m_t[:, :],
                                      op=ALU.max)
                else:  # "M2": Prelu(alpha~0) keeps the masked marker; no TT
                    sm_t = sm_pool.tile([128, W], BF16, tag="smt")
                    nc.sync.dma_start(sm_t[:, :],
                                      smm[mi * 128:(mi + 1) * 128, :])
                    mi += 1
                    r_t = r_pool.tile([128, W], F32, tag="rt")
                    nc.scalar.activation(r_t[:, :], sm_t[:, :], AF.Prelu,
                                         bias=dc_sb[h][:, jc:jc + 1],
                                         alpha=1e-4)
                    nc.scalar.activation(p_t[:, :], r_t[:, :], AF.Exp,
                                         scale=0.8)
                for q in range(W // 512):
                    nc.tensor.matmul(ps_O[h][:, q * 512:(q + 1) * 512],
                                     vt_sb[h][:, jc * VW:(jc + 1) * VW],
                                     p_t[:, q * 512:(q + 1) * 512],
                                     start=(jc == 0), stop=(jc == NCH - 1))

        for h in range(2):
            o_t = const_pool.tile([VW, W], F32, tag=f"ot{h}", name=f"ot{h}")
            nc.scalar.copy(o_t[:, :], ps_O[h][:, :])
            nc.sync.dma_start(outT[h][:, :], o_t[:, :])
    nc.compile()
    return nc


_CACHED_NC = None


def _get_nc():
    global _CACHED_NC
    if _CACHED_NC is None:
        _CACHED_NC = build_program()
    return _CACHED_NC


def _bf(x):
    return np.ascontiguousarray(x.astype(ml_dtypes.bfloat16))


def _prep_inputs(h, adj, w, a_src, a_dst, b):
    h = np.asarray(h, dtype=np.float32)
    adj = np.asarray(adj)
    w = np.asarray(w, dtype=np.float32)
    a_src = np.asarray(a_src, dtype=np.float32)
    a_dst = np.asarray(a_dst, dtype=np.float32)
    b = np.asarray(b, dtype=np.float32)

    adjT = adj.T  # [j, i] layout
    # per global head: s, d, V
    s_all, d_all, vt_all = [], [], []
    for g in range(N_HEAD):
        s = h @ (w[g] @ a_src[g])[:, 0]             # [N] per-row logit
        d = h @ (w[g] @ a_dst[g])[:, 0]             # [N] per-col logit
        V = h @ w[g] + b[None, :]                   # [N, F_OUT]
        v2 = np.exp(NEG * d)                        # e^{0.2 d}
        vt = np.concatenate([V * v2[:, None], v2[:, None]], axis=1)  # [N, VW]
        s_all.append(s)
        d_all.append(d)
        vt_all.append(vt)

    in_maps = []
    for c in range(N_HEAD):
        pair, half = c % 4, c // 4
        isl = slice(half * W, (half + 1) * W)
        adjT_sl = adjT[:, isl]                      # [N, W] bool
        mp = {"maskT": _bf(adjT_sl.astype(np.float32))}
        smm_blocks = []
        for hh in range(2):
            g = 2 * pair + hh
            s = s_all[g]
            d = d_all[g]
            s_sl = s[isl].astype(np.float32)
            mp[f"u8b{hh}"] = _bf(np.broadcast_to(
                np.exp(0.8 * s_sl)[None, :], (128, W)))
            mp[f"sbb{hh}"] = _bf(np.broadcast_to(s_sl[None, :], (128, W)))
            vt128 = vt_all[g].reshape(NCH, 128, VW).transpose(1, 0, 2)
            mp[f"vt{hh}"] = _bf(vt128.reshape(128, NCH * VW))
            dcol = d.reshape(NCH, 128).T.astype(np.float32)     # [128, NCH]
            mp[f"v8c{hh}"] = np.ascontiguousarray(np.exp(0.8 * dcol))
            mp[f"dc{hh}"] = np.ascontiguousarray(dcol)
            mp[f"d8c{hh}"] = np.ascontiguousarray(0.8 * dcol)
        # SM blocks for M-route slots, in device emission order
        s_bf_sl = [None, None]
        for k, r in enumerate(ROUTES):
            if r not in SM_ROUTES:
                continue
            jc, hh = k // 2, k % 2
            g = 2 * pair + hh
            if s_bf_sl[hh] is None:
                s_bf_sl[hh] = s_all[g][isl].astype(np.float32)
            blk = np.where(adjT_sl[jc * 128:(jc + 1) * 128, :],
                           s_bf_sl[hh][None, :], np.float32(MASKED))
            smm_blocks.append(blk)
        if smm_blocks:
            mp["smm"] = _bf(np.concatenate(smm_blocks, axis=0))
        else:
            mp["smm"] = _bf(np.zeros((128, W), np.float32))
        in_maps.append(mp)
    return in_maps


def _run(in_maps, trace=False, **kwargs):
    nc = _get_nc()
    return run_bass_kernel_spmd(nc, in_maps, list(range(N_HEAD)), trace=trace,
                                **kwargs)


def _assemble(res):
    out = np.empty((N_HEAD, N, F_OUT), dtype=np.float32)
    for c in range(N_HEAD):
        pair, half = c % 4, c // 4
        isl = slice(half * W, (half + 1) * W)
        for hh in range(2):
            g = 2 * pair + hh
            blk = np.asarray(res.results[c][f"outT{hh}"], dtype=np.float32)
            out[g, isl, :] = (blk[:F_OUT, :] / blk[F_OUT:VW, :]).T
    return out


def kernel(h, adj, w, a_src, a_dst, b):
    in_maps = _prep_inputs(h, adj, w, a_src, a_dst, b)
    res = _run(in_maps)
    return _assemble(res)


# revision 15
# speedup vs baseline: 4.0267x; 1.0772x over previous
"""Trainium2 Bass kernel for nn_MultiHeadGraphAttention (v5).

Multi-head graph attention (GAT-style):
    h_prime = einsum('nf,hfo->hno', h, w)
    attn    = softmax(where(adj, leakyrelu(s_i + d_j), -inf), axis=-1)
    out     = attn @ h_prime + b
with s = h_prime @ a_src, d = h_prime @ a_dst, n=4096, H=8, f_out=64.

Key identity:  exp(leakyrelu(x)) = e^{0.2x} * max(e^{0.8x}, 1)
For x = s_i + d_j the e^{0.2x} factor splits into e^{0.2 s_i} (per attention
row i => cancels in the softmax normalization, done on host) and e^{0.2 d_j}
(per contraction index j => folded into the matmul stationary operand
Vt = [V*v2 | v2] on host).  The device only computes, per j-chunk,
    p[j,i] = max(e^{0.8(s_i+d_j)}, 1) * m[j,i]
and accumulates out^T = Vt^T @ p in PSUM (the ones column of Vt yields the
softmax denominators for free).  Final normalize + transpose on host.

Sharding: 8 cores = 4 head-pairs x 2 column-halves.  Each core computes two
heads over a 2048-wide slice of attention rows i; the adjacency mask slice
(bf16 [4096, 2048]) is shared by both heads.

The run is DMA-bandwidth-bound, so bytes are packed aggressively:
 - mask chunks are packed into multi-chunk groups with 12KB contiguous rows
 - 8 j-chunks are "M2" for both heads: their p comes entirely from ScalarE
   via a host-built SM tensor (s_i, masked entries = -3e5), so those chunks
   need NO mask bytes at all
 - per-head constants ride in two packed DMAs

Per-slot routes (slot = (j-chunk, head)):
  V : DVE tensor_scalar 4x  t = (U8 * v8_j) max 1 ; p = t * m  (TT 2x)
  A : ACT r = Relu(S_b + d_j); e = Exp(0.8 r)     ; p = e * m  (TT 2x)
  M2: ACT r = Prelu(SM + d_j, a=1e-4); p = Exp(0.8 r)   -- no DVE, no mask;
      the -3e5 masked marker survives the Prelu kink (r=-30 => p=e^-24~0)
"""
import sys

if "/opt/trn_rl_repo" not in sys.path:
    sys.path.insert(0, "/opt/trn_rl_repo")

from contextlib import ExitStack

import ml_dtypes
import numpy as np

import concourse.bass as bass
import concourse.bacc as bacc
import concourse.tile as tile
from concourse import mybir
from concourse.bass_utils import run_bass_kernel_spmd

F32 = mybir.dt.float32
BF16 = mybir.dt.bfloat16
AF = mybir.ActivationFunctionType
ALU = mybir.AluOpType

N = 4096
F_IN = 256
N_HEAD = 8
F_OUT = 64
NEG = 0.2
W = 2048              # attention-row (i) slice width per core
NCH = N // 128        # 32 j-chunks
VW = F_OUT + 1        # 65: V columns + ones (denominator) column
MASKED = -3e5         # masked marker; survives Prelu(alpha=1e-4)

# both slots of these j-chunks take the M2 route (no mask bytes needed)
M2_JCS = (3, 7, 11, 15, 19, 23, 27, 31)
# (jc, h) slots on the A route (2xACT + mask mult)
A_SLOTS = ((1, 0), (5, 0), (9, 0), (13, 0), (17, 0), (21, 0))
# mask chunk groups packed into single DMAs (contiguous rows)
MASK_GROUPS = ([0], [1, 2], [4, 5, 6], [8, 9, 10], [12, 13, 14],
               [16, 17, 18], [20, 21, 22], [24, 25, 26], [28, 29, 30])
JC2GROUP = {}
for _g, _jcs in enumerate(MASK_GROUPS):
    for _o, _jc in enumerate(_jcs):
        JC2GROUP[_jc] = (_g, _o)


def _route(jc, h):
    if jc in M2_JCS:
        return "M2"
    if (jc, h) in A_SLOTS:
        return "A"
    return "V"


def build_program():
    nc = bacc.Bacc("TRN2", target_bir_lowering=False, debug=False)
    maskg = [nc.dram_tensor(f"maskg{g}", [128, len(jcs) * W], BF16,
                            kind="ExternalInput").ap()
             for g, jcs in enumerate(MASK_GROUPS)]
    smp = [nc.dram_tensor(f"smp{i}", [128, 2 * W], BF16,
                          kind="ExternalInput").ap()
           for i in range(len(M2_JCS))]
    cb = [nc.dram_tensor(f"cb{h}", [128, 2 * W], BF16,
                         kind="ExternalInput").ap() for h in range(2)]
    cf = [nc.dram_tensor(f"cf{h}", [128, 3 * NCH], F32,
                         kind="ExternalInput").ap() for h in range(2)]
    vt = [nc.dram_tensor(f"vt{h}", [128, NCH * VW], BF16,
                         kind="ExternalInput").ap() for h in range(2)]
    outT = [nc.dram_tensor(f"outT{h}", [VW, W], F32,
                           kind="ExternalOutput").ap() for h in range(2)]

    with tile.TileContext(nc) as tc, ExitStack() as ctx:
        const_pool = ctx.enter_context(tc.tile_pool(name="const", bufs=1))
        mask_pool = ctx.enter_context(tc.tile_pool(name="mask", bufs=2))
        sm_pool = ctx.enter_context(tc.tile_pool(name="sm", bufs=2))
        t_pool = ctx.enter_context(tc.tile_pool(name="tw", bufs=3))
        r_pool = ctx.enter_context(tc.tile_pool(name="rw", bufs=2))
        e_pool = ctx.enter_context(tc.tile_pool(name="ew", bufs=3))
        p_pool = ctx.enter_context(tc.tile_pool(name="pw", bufs=6))
        ps_pool = ctx.enter_context(tc.tile_pool(name="ps", bufs=1, space="PSUM"))

        # ---- constants: h0 first, interleaved with the first mask group ----
        cb_sb, cf_sb, vt_sb = [], [], []
        cbt0 = const_pool.tile([128, 2 * W], BF16, tag="cb0")
        nc.sync.dma_start(cbt0[:, :], cb[0][:, :])
        g0 = mask_pool.tile([128, 3 * W], BF16, tag="mg", name="mg0")
        nc.sync.dma_start(g0[:, 0:len(MASK_GROUPS[0]) * W], maskg[0][:, :])
        cft0 = const_pool.tile([128, 3 * NCH], F32, tag="cf0")
        nc.sync.dma_start(cft0[:, :], cf[0][:, :])
        vtt0 = const_pool.tile([128, NCH * VW], BF16, tag="vt0")
        nc.sync.dma_start(vtt0[:, :], vt[0][:, :])
        cbt1 = const_pool.tile([128, 2 * W], BF16, tag="cb1")
        nc.sync.dma_start(cbt1[:, :], cb[1][:, :])
        cft1 = const_pool.tile([128, 3 * NCH], F32, tag="cf1")
        nc.sync.dma_start(cft1[:, :], cf[1][:, :])
        vtt1 = const_pool.tile([128, NCH * VW], BF16, tag="vt1")
        nc.sync.dma_start(vtt1[:, :], vt[1][:, :])
        cb_sb = [cbt0, cbt1]
        cf_sb = [cft0, cft1]
        vt_sb = [vtt0, vtt1]
        # views into the packed consts
        u8_sb = [t[:, 0:W] for t in cb_sb]
        sb_sb = [t[:, W:2 * W] for t in cb_sb]
        v8_sb = [t[:, 0:NCH] for t in cf_sb]
        dc_sb = [t[:, NCH:2 * NCH] for t in cf_sb]

        ps_O = [ps_pool.tile([VW, W], F32, tag=f"psO{h}", name=f"psO{h}")
                for h in range(2)]

        # ---- attention j-loop ----
        group_tiles = {0: g0}
        sm_tiles = {}
        mi = 0
        for jc in range(NCH):
            # prefetch the SM pair for this 4-chunk period's M2 jc
            if jc % 4 == 0:
                m2jc = jc + 3
                si = M2_JCS.index(m2jc)
                sm_t = sm_pool.tile([128, 2 * W], BF16, tag="smt",
                                    name=f"sm{si}")
                nc.sync.dma_start(sm_t[:, :], smp[si][:, :])
                sm_tiles[m2jc] = sm_t
            if jc in JC2GROUP:
                g, off = JC2GROUP[jc]
                if g not in group_tiles:
                    gt = mask_pool.tile([128, 3 * W], BF16, tag="mg",
                                        name=f"mg{g}")
                    nc.sync.dma_start(gt[:, 0:len(MASK_GROUPS[g]) * W],
                                      maskg[g][:, :])
                    group_tiles[g] = gt
                m_t = group_tiles[JC2GROUP[jc][0]][:, off * W:(off + 1) * W]
            else:
                m_t = None
            for h in range(2):
                r = _route(jc, h)
                p_t = p_pool.tile([128, W], BF16, tag="pt")
                if r == "V":
                    t_t = t_pool.tile([128, W], BF16, tag="tt")
                    nc.vector.tensor_scalar(t_t[:, :], u8_sb[h],
                                            v8_sb[h][:, jc:jc + 1], 1.0,
                                            op0=ALU.mult, op1=ALU.max)
                    nc.vector.tensor_tensor(p_t[:, :], t_t[:, :], m_t,
                                            op=ALU.mult)
                elif r == "A":
                    r_t = r_pool.tile([128, W], F32, tag="rt")
                    nc.scalar.activation(r_t[:, :], sb_sb[h], AF.Relu,
                                         bias=dc_sb[h][:, jc:jc + 1])
                    e_t = e_pool.tile([128, W], BF16, tag="et")
                    nc.scalar.activation(e_t[:, :], r_t[:, :], AF.Exp,
                                         scale=0.8)
                    nc.vector.tensor_tensor(p_t[:, :], e_t[:, :], m_t,
                                            op=ALU.mult)
                else:  # "M2"
                    sm_t = sm_tiles[jc]
                    r_t = r_pool.tile([128, W], F32, tag="rt")
                    nc.scalar.activation(r_t[:, :], sm_t[:, h * W:(h + 1) * W],
                                         AF.Prelu,
                                         bias=dc_sb[h][:, jc:jc + 1],
                                         alpha=1e-4)
                    nc.scalar.activation(p_t[:, :], r_t[:, :], AF.Exp,
                                         scale=0.8)
                for q in range(W // 512):
                    nc.tensor.matmul(ps_O[h][:, q * 512:(q + 1) * 512],
                                     vt_sb[h][:, jc * VW:(jc + 1) * VW],
                                     p_t[:, q * 512:(q + 1) * 512],
                                     start=(jc == 0), stop=(jc == NCH - 1))

        for h in range(2):
            o_t = const_pool.tile([VW, W], F32, tag=f"ot{h}", name=f"ot{h}")
            nc.scalar.copy(o_t[:, :], ps_O[h][:, :])
            nc.sync.dma_start(outT[h][:, :], o_t[:, :])
    nc.compile()
    return nc


_CACHED_NC = None


def _get_nc():
    global _CACHED_NC
    if _CACHED_NC is None:
        _CACHED_NC = build_program()
    return _CACHED_NC


def _bf(x):
    return np.ascontiguousarray(x.astype(ml_dtypes.bfloat16))


def _prep_inputs(h, adj, w, a_src, a_dst, b):
    h = np.asarray(h, dtype=np.float32)
    adj = np.asarray(adj)
    w = np.asarray(w, dtype=np.float32)
    a_src = np.asarray(a_src, dtype=np.float32)
    a_dst = np.asarray(a_dst, dtype=np.float32)
    b = np.asarray(b, dtype=np.float32)

    adjT = adj.T  # [j, i] layout
    s_all, d_all, vt_all = [], [], []
    for g in range(N_HEAD):
        s = h @ (w[g] @ a_src[g])[:, 0]
        d = h @ (w[g] @ a_dst[g])[:, 0]
        V = h @ w[g] + b[None, :]
        v2 = np.exp(NEG * d)
        vt_all.append(np.concatenate([V * v2[:, None], v2[:, None]], axis=1))
        s_all.append(s)
        d_all.append(d)

    in_maps = []
    for c in range(N_HEAD):
        pair, half = c % 4, c // 4
        isl = slice(half * W, (half + 1) * W)
        adjT_sl = adjT[:, isl]                      # [N, W] bool
        mp = {}
        mf = _bf(adjT_sl.astype(np.float32))
        for g, jcs in enumerate(MASK_GROUPS):
            mp[f"maskg{g}"] = np.ascontiguousarray(np.concatenate(
                [mf[jc * 128:(jc + 1) * 128, :] for jc in jcs], axis=1))
        s_sl, d_col = [], []
        for hh in range(2):
            gh = 2 * pair + hh
            s = s_all[gh][isl].astype(np.float32)
            d = d_all[gh]
            s_sl.append(s)
            dcol = d.reshape(NCH, 128).T.astype(np.float32)
            d_col.append(dcol)
            u8 = np.broadcast_to(np.exp(0.8 * s)[None, :], (128, W))
            sb = np.broadcast_to(s[None, :], (128, W))
            mp[f"cb{hh}"] = _bf(np.concatenate([u8, sb], axis=1))
            mp[f"cf{hh}"] = np.ascontiguousarray(np.concatenate(
                [np.exp(0.8 * dcol), dcol, 0.8 * dcol], axis=1,
                dtype=np.float32))
            vt128 = vt_all[gh].reshape(NCH, 128, VW).transpose(1, 0, 2)
            mp[f"vt{hh}"] = _bf(vt128.reshape(128, NCH * VW))
        for si, jc in enumerate(M2_JCS):
            blocks = []
            for hh in range(2):
                blocks.append(np.where(adjT_sl[jc * 128:(jc + 1) * 128, :],
                                       s_sl[hh][None, :], np.float32(MASKED)))
            mp[f"smp{si}"] = _bf(np.concatenate(blocks, axis=1))
        in_maps.append(mp)
    return in_maps


def _run(in_maps, trace=False, **kwargs):
    nc = _get_nc()
    return run_bass_kernel_spmd(nc, in_maps, list(range(N_HEAD)), trace=trace,
                                **kwargs)


def _assemble(res):
    out = np.empty((N_HEAD, N, F_OUT), dtype=np.float32)
    for c in range(N_HEAD):
        pair, half = c % 4, c // 4
        isl = slice(half * W, (half + 1) * W)
        for hh in range(2):
            g = 2 * pair + hh
            blk = np.asarray(res.results[c][f"outT{hh}"], dtype=np.float32)
            out[g, isl, :] = (blk[:F_OUT, :] / blk[F_OUT:VW, :]).T
    return out


def kernel(h, adj, w, a_src, a_dst, b):
    in_maps = _prep_inputs(h, adj, w, a_src, a_dst, b)
    res = _run(in_maps)
    return _assemble(res)
